# Initial kernel scaffold
#
"""Multi-head attention (B=4, N=2048, C=1024, H=16, D=64) on 8 Trainium2 cores.

Sharding: core = b*2 + hg  (b in 0..3 batches, hg in 0..1 head-groups of 8 heads).
Each core computes, for its (b, hg):
    Q^T, K^T   [512, 2048]  (8 heads x 64 dims on partitions, queries on free)
    V          [2048, 512]  (keys on partitions) + ones column (softmax denom)
    per head pair (2 heads stacked on 128 partitions):
        S^T tiles = K_h^T.T @ Q_h^T  (keys on partitions, queries free)
        expS^T = exp(SCALE * S^T)    (ScalarE, scale folded into activation)
        ctx^T/den = [V_h | 1].T @ expS^T   (ones-augmented PV matmul: row 64 = denom)
        normalize: den -> DRAM -> stride-0 broadcast DMA -> fast reciprocal -> mul
    out_partial = ctx^T.T @ wo_hg^T  [2048, 1024]
Host: out[b] = out_partial[b,hg=0] + out_partial[b,hg=1] + bo.

All matmuls run in float32r (full-speed fp32-reduced mode, ~1e-4 rel err).
S^T matmuls for the two heads of a pair are row-packed onto the 128x128 PE
array (K=64 each at base partitions 0/64) and run concurrently.
"""
import sys

sys.path.insert(0, "/opt/trn_rl_repo")

import numpy as np

import concourse.bass as bass  # noqa: F401
import concourse.tile as tile
from concourse import bacc, mybir
from concourse.bass_utils import run_bass_kernel_spmd

P = 128
B, N, C = 4, 2048, 1024
H = 16
D = 64
HG = 2                 # head groups (tensor-parallel dim)
NH = H // HG           # 8 heads per core
CH = NH * D            # 512 ctx channels per core
KO = C // P            # 8 contraction tiles for projections
NKT = N // P           # 16 key tiles
QC = 512               # query chunk (psum bank)
NQC = N // QC          # 4 query chunks
KTG = 2                # key tiles per exp group
NXQ = 4                # x streamed in quarters
SCALE = D ** -0.5

f32 = mybir.dt.float32
f32r = mybir.dt.float32r

_CACHE = {}


def _build(variant="full"):
    """Build + compile the per-core Bass program (same for all 8 cores).

    variant: "full" | "nop" (overhead probe) | "qkv" (phase 1 only) |
             "attn" (phases 1+2, no projection) — for ablation benchmarking.
    """
    if variant in _CACHE:
        return _CACHE[variant]

    nc = bacc.Bacc("TRN2", target_bir_lowering=False, debug=False)

    xt_d = nc.dram_tensor("xt", [KO, P, N], f32r, kind="ExternalInput").ap()
    wq_d = nc.dram_tensor("wq", [KO, P, CH], f32r, kind="ExternalInput").ap()
    wk_d = nc.dram_tensor("wk", [KO, P, CH], f32r, kind="ExternalInput").ap()
    wv_d = nc.dram_tensor("wv", [KO, P, CH], f32r, kind="ExternalInput").ap()
    wo_d = nc.dram_tensor("wo", [CH // P, P, C], f32r, kind="ExternalInput").ap()
    ones_d = nc.dram_tensor("ones", [P, 1], f32r, kind="ExternalInput").ap()
    out_d = nc.dram_tensor("out", [N, C], f32, kind="ExternalOutput").ap()

    with tile.TileContext(nc) as tc:
        with tc.tile_pool(name="persist", bufs=1) as persist:
            if variant == "nop":
                nop_t = persist.tile([P, QC], f32, tag="nop")
                nc.vector.memset(nop_t[:], 0.0)
                nc.sync.dma_start(out_d[0:P, 0:QC], nop_t[:])
            else:
                _build_body(nc, tc, persist, variant,
                            xt_d, wq_d, wk_d, wv_d, wo_d, ones_d, out_d)

    nc.compile()
    _CACHE[variant] = nc
    return nc


def _build_body(nc, tc, persist, variant, xt_d, wq_d, wk_d, wv_d, wo_d, ones_d, out_d):
    qt = persist.tile([P, CH // P, N], f32r, tag="qt")
    kt = persist.tile([P, CH // P, N], f32r, tag="kt")
    v = persist.tile([P, NKT, NH, D + 1], f32r, tag="v")
    ctxt = persist.tile([P, CH // P, N], f32r, tag="ctxt")
    ones = persist.tile([P, 1], f32r, tag="ones")
    nc.sync.dma_start(ones[:], ones_d[:])

    # ---------------- phase 1: QKV projections ----------------
    # x^T streamed in query-quarters to fit SBUF; weights q/k/v resident.
    NQUARTER = N // NXQ
    with (
        tc.tile_pool(name="px", bufs=1) as px,
        tc.tile_pool(name="pw", bufs=1) as pw,
        tc.tile_pool(name="psum1", bufs=4, space="PSUM") as psum1,
    ):
        wts = {}
        for name, wd in (("v", wv_d), ("q", wq_d), ("k", wk_d)):
            w = pw.tile([P, KO, CH], f32r, tag=f"w{name}")
            for ko in range(KO):
                nc.sync.dma_start(w[:, ko, :], wd[ko])
            wts[name] = w

        for quarter in range(NXQ):
            hsl = slice(quarter * NQUARTER, (quarter + 1) * NQUARTER)
            xt = px.tile([P, KO, NQUARTER], f32r, tag="x")
            for ko in range(KO):
                nc.sync.dma_start(xt[:, ko, :], xt_d[ko, :, hsl])

            # V first (attention needs all of V before any PV work)
            for i in range(NQUARTER // P):
                ikt = quarter * (NQUARTER // P) + i
                ps = psum1.tile([P, CH], f32, tag="ps1")
                for ko in range(KO):
                    nc.tensor.matmul(
                        ps[:], xt[:, ko, i * P:(i + 1) * P], wts["v"][:, ko, :],
                        start=(ko == 0), stop=(ko == KO - 1))
                nc.vector.tensor_copy(
                    v[:, ikt, :, 0:D], ps[:].rearrange("p (h d) -> p h d", d=D))

            # Q, K per m-tile (head pair)
            for mt in range(CH // P):
                for name, dst in (("q", qt), ("k", kt)):
                    for i in range(NQUARTER // QC):
                        qc = quarter * (NQUARTER // QC) + i
                        ps = psum1.tile([P, QC], f32, tag="ps1")
                        for ko in range(KO):
                            nc.tensor.matmul(
                                ps[:],
                                wts[name][:, ko, mt * P:(mt + 1) * P],
                                xt[:, ko, i * QC:(i + 1) * QC],
                                start=(ko == 0), stop=(ko == KO - 1))
                        nc.vector.tensor_copy(dst[:, mt, qc * QC:(qc + 1) * QC], ps[:])
    # ones column (col 64) for all key tiles / heads in one op
    nc.vector.tensor_copy(
        v[:, :, :, D:D + 1],
        ones[:].unsqueeze(1).unsqueeze(1).broadcast_to([P, NKT, NH, 1]))

    if variant == "qkv":
        ot = persist.tile([P, QC], f32, tag="dump")
        nc.vector.tensor_copy(
            ot[:].rearrange("p (h d) -> p h d", d=D), v[:, 0, :, 0:D].bitcast(f32))
        nc.sync.dma_start(out_d[0:P, 0:QC], ot[:])
        return

    # ---------------- phases 2+3: attention with interleaved projection ----
    # qc-outer / head-pair-inner; after each 512-query block's attention
    # completes for all head pairs, its output projection is emitted so the
    # PE-only projection work fills the ScalarE-bound attention slack.
    # Projection PSUM accumulators share the ppv pool slots (same shape/tag).
    with (
        tc.tile_pool(name="pwo", bufs=1) as pwo,
        tc.tile_pool(name="pe", bufs=2) as pe_pool,
        tc.tile_pool(name="pden", bufs=2) as pden,
        tc.tile_pool(name="pout", bufs=4) as pout,
        tc.tile_pool(name="pdram", bufs=2, space="DRAM") as pdram,
    ):
        wo = pwo.tile([P, CH // P, C], f32r, tag="wo")
        for ct in range(CH // P):
            nc.sync.dma_start(wo[:, ct, :], wo_d[ct])

        with (
            tc.tile_pool(name="pst", bufs=1, space="PSUM") as pst,
            tc.tile_pool(name="ppv", bufs=2, space="PSUM") as ppv,
        ):
            for qc in range(NQC):
                qsl = slice(qc * QC, (qc + 1) * QC)
                for hp in range(CH // P):    # head pair = partition tile of qt/kt
                    psA = ppv.tile([P, QC], f32, tag="pvA")
                    psB = ppv.tile([P, QC], f32, tag="pvB")
                    for g in range(NKT // KTG):
                        stA = pst.tile([P, KTG, QC], f32, tag="stA")
                        stB = pst.tile([P, KTG, QC], f32, tag="stB")
                        for j in range(KTG):
                            ik = g * KTG + j
                            ksl = slice(ik * P, (ik + 1) * P)
                            nc.tensor.matmul(stA[:, j, :], kt[0:D, hp, ksl],
                                             qt[0:D, hp, qsl], start=True, stop=True)
                            nc.tensor.matmul(stB[:, j, :], kt[D:P, hp, ksl],
                                             qt[D:P, hp, qsl], start=True, stop=True)
                        eA = pe_pool.tile([P, KTG, QC], f32r, tag="eA")
                        eB = pe_pool.tile([P, KTG, QC], f32r, tag="eB")
                        nc.scalar.activation(eA[:], stA[:],
                                             mybir.ActivationFunctionType.Exp, scale=SCALE)
                        nc.scalar.activation(eB[:], stB[:],
                                             mybir.ActivationFunctionType.Exp, scale=SCALE)
                        for j in range(KTG):
                            ik = g * KTG + j
                            first, last = ik == 0, ik == NKT - 1
                            nc.tensor.matmul(psA[0:D + 1, :], v[:, ik, 2 * hp, :],
                                             eA[:, j, :], start=first, stop=last)
                            nc.tensor.matmul(psB[0:D + 1, :], v[:, ik, 2 * hp + 1, :],
                                             eB[:, j, :], start=first, stop=last)
                    # softmax denominators: stage to DRAM (same-partition copy
                    # first), broadcast back across partitions via stride-0
                    # DMA, batched fast reciprocal, then normalize.
                    den = pden.tile([P, 2, QC], f32, tag="den")
                    nc.vector.tensor_copy(den[D:D + 1, 0, :], psA[D:D + 1, :])
                    nc.vector.tensor_copy(den[D:D + 1, 1, :], psB[D:D + 1, :])
                    den_dr = pdram.tile([2, QC], f32, tag="den_dr")
                    nc.sync.dma_start(den_dr[:], den[D:D + 1, :, :])
                    bcr = pden.tile([P, QC], f32, tag="bcr")
                    nc.sync.dma_start(bcr[0:D, :], den_dr[0].partition_broadcast(D))
                    nc.sync.dma_start(bcr[D:P, :], den_dr[1].partition_broadcast(D))
                    bc = pden.tile([P, QC], f32, tag="bc")
                    nc.vector.reciprocal_approx_fast(bc[:], bcr[:])
                    nc.vector.tensor_mul(ctxt[0:D, hp, qsl], psA[0:D, :], bc[0:D, :])
                    nc.vector.tensor_mul(ctxt[D:P, hp, qsl], psB[0:D, :], bc[D:P, :])

                if variant == "attn":
                    continue
                # projection for this 512-query block (PSUM slots shared with pvA)
                for qt_i in range(4 * qc, 4 * qc + 4):
                    for nt in range(C // QC):
                        ps = ppv.tile([P, QC], f32, tag="pvA")
                        for ct in range(CH // P):
                            nc.tensor.matmul(
                                ps[:], ctxt[:, ct, qt_i * P:(qt_i + 1) * P],
                                wo[:, ct, nt * QC:(nt + 1) * QC],
                                start=(ct == 0), stop=(ct == CH // P - 1))
                        ot = pout.tile([P, QC], f32, tag="ot")
                        nc.vector.tensor_copy(ot[:], ps[:])
                        nc.sync.dma_start(
                            out_d[qt_i * P:(qt_i + 1) * P, nt * QC:(nt + 1) * QC], ot[:])

        if variant == "attn":
            ot = persist.tile([P, QC], f32, tag="dump")
            nc.vector.tensor_copy(ot[:], ctxt[:, 0, 0:QC].bitcast(f32))
            nc.sync.dma_start(out_d[0:P, 0:QC], ot[:])


def _prepare_in_maps(x, wq, wk, wv, wo):
    x = np.ascontiguousarray(np.asarray(x, dtype=np.float32))
    ws = {}
    for hg in range(HG):
        sl = slice(hg * CH, (hg + 1) * CH)
        ws[hg] = {
            "wq": np.ascontiguousarray(np.asarray(wq)[sl, :].T).reshape(KO, P, CH),
            "wk": np.ascontiguousarray(np.asarray(wk)[sl, :].T).reshape(KO, P, CH),
            "wv": np.ascontiguousarray(np.asarray(wv)[sl, :].T).reshape(KO, P, CH),
            "wo": np.ascontiguousarray(np.asarray(wo)[:, sl].T).reshape(CH // P, P, C),
        }
    ones = np.ones((P, 1), dtype=np.float32)
    in_maps = []
    for core in range(8):
        b, hg = core // HG, core % HG
        xt = np.ascontiguousarray(x[b].T).reshape(KO, P, N)
        m = {"xt": xt, "ones": ones}
        m.update(ws[hg])
        in_maps.append(m)
    return in_maps


def kernel(x, wq, wk, wv, wo, bo):
    nc = _build()
    in_maps = _prepare_in_maps(x, wq, wk, wv, wo)
    res = run_bass_kernel_spmd(nc, in_maps, core_ids=list(range(8)))
    bo = np.asarray(bo, dtype=np.float32)
    out = np.empty((B, N, C), dtype=np.float32)
    for b in range(B):
        out[b] = res.results[2 * b]["out"] + res.results[2 * b + 1]["out"] + bo
    return out



# revision 2
# speedup vs baseline: 1.1232x; 1.1232x over previous
"""Multi-head attention (B=4, N=2048, C=1024, H=16, D=64) on 8 Trainium2 cores.

Sharding: core = b*2 + hg  (b in 0..3 batches, hg in 0..1 head-groups of 8 heads).

v3 — all-bf16 datapath (f32 PSUM accumulation, rel err ~4e-3):
  - qt/kt [128 = 2 heads x 64 d, 4 head-pairs, 2048] bf16; S^T per head via
    K=64 matmuls row-positioned at partition 0/64 (tile_position).
  - per-(qc, head) S psum [128 keys, 2 ktile, 512 q], double-buffered so
    S(g+1) overlaps exp(g) — exp is one Act instruction per [128, 1024].
  - PV FLIPPED: stationary = expS tile [128 keys, 128 q], moving = v_aug
    [128 keys, 65] (64 ch + ones) -> psum [128 q, 4 qsub, 65]; col 64 =
    softmax denominator, per-partition. 65-cycle matmuls instead of 512.
  - normalize: DVE reciprocal [128,4,1] + broadcast-free tensor_mul (stride-0
    free-dim read), then PE transpose (identity matmul) back to [ch, tokens]
    for the output projection; Pool copies transpose psum -> ctxt bf16.
  - projection work (K mt1-3, V, most of Q, output proj) is DRAINED into the
    attention stream so the Act engine starts exp after a ~10 us prefix
    (K mt0 + Q mt0/qc0) instead of a ~60 us serial QKV phase.
"""
import os
import sys

sys.path.insert(0, "/opt/trn_rl_repo")

import ml_dtypes
import numpy as np

import concourse.bass as bass  # noqa: F401
import concourse.tile as tile
from concourse import bacc, mybir
from concourse.bass_utils import run_bass_kernel_spmd

P = 128
B, N, C = 4, 2048, 1024
H = 16
D = 64
HG = 2                 # head groups (tensor-parallel dim)
NH = H // HG           # 8 heads per core
CH = NH * D            # 512 ctx channels per core
KO = C // P            # 8 contraction tiles for projections
NKT = N // P           # 16 key tiles
QC = 512               # query chunk (psum bank)
NQC = N // QC          # 4 query chunks
NG = NKT // 2          # 8 key-tile pair groups per (qc, head)
SCALE = D ** -0.5

f32 = mybir.dt.float32
bf16 = mybir.dt.bfloat16

np_bf16 = ml_dtypes.bfloat16

BCAST = os.environ.get("BCAST", "dram")  # "pool" | "dram"

_CACHE = {}


def _build(variant="full"):
    """Build + compile the per-core Bass program (same for all 8 cores)."""
    key = (variant, BCAST)
    if key in _CACHE:
        return _CACHE[key]

    nc = bacc.Bacc("TRN2", target_bir_lowering=False, debug=False)

    xt_d = nc.dram_tensor("xt", [KO, P, N], bf16, kind="ExternalInput").ap()
    wq_d = nc.dram_tensor("wq", [KO, P, CH], bf16, kind="ExternalInput").ap()
    wk_d = nc.dram_tensor("wk", [KO, P, CH], bf16, kind="ExternalInput").ap()
    wv_d = nc.dram_tensor("wv", [KO, P, CH], bf16, kind="ExternalInput").ap()
    wo_d = nc.dram_tensor("wo", [CH // P, P, C], bf16, kind="ExternalInput").ap()
    eye_d = nc.dram_tensor("eye", [P, P], bf16, kind="ExternalInput").ap()
    out_d = nc.dram_tensor("out", [N, C], f32, kind="ExternalOutput").ap()

    with tile.TileContext(nc) as tc:
        with tc.tile_pool(name="persist", bufs=1) as persist:
            _build_body(nc, tc, persist, variant,
                        xt_d, wq_d, wk_d, wv_d, wo_d, eye_d, out_d)

    nc.compile()
    _CACHE[key] = nc
    return nc


def _build_body(nc, tc, persist, variant, xt_d, wq_d, wk_d, wv_d, wo_d, eye_d,
                out_d):
    xt = persist.tile([P, KO, N], bf16, tag="xt")
    qt = persist.tile([P, CH // P, N], bf16, tag="qt")
    kt = persist.tile([P, CH // P, N], bf16, tag="kt")
    v = persist.tile([P, NKT, NH, D + 1], bf16, tag="v")
    ctxt = persist.tile([P, CH // P, N], bf16, tag="ctxt")
    wq = persist.tile([P, KO, CH], bf16, tag="wq")
    wk = persist.tile([P, KO, CH], bf16, tag="wk")
    wv = persist.tile([P, KO, CH], bf16, tag="wv")
    wo = persist.tile([P, CH // P, C], bf16, tag="wo")
    eye = persist.tile([P, P], bf16, tag="eye")
    nc.sync.dma_start(eye[:], eye_d[:])

    # DMA order mirrors first-use order: the prefix K projection reads
    # xt[ko] + wk[ko] in ko order; wq next (Q prefix), wv (V drains), wo last.
    for ko in range(KO):
        nc.sync.dma_start(xt[:, ko, :], xt_d[ko])
        nc.sync.dma_start(wk[:, ko, :], wk_d[ko])
    for ko in range(KO):
        nc.sync.dma_start(wq[:, ko, :], wq_d[ko])
    for ko in range(KO):
        nc.sync.dma_start(wv[:, ko, :], wv_d[ko])
    for ct in range(CH // P):
        nc.sync.dma_start(wo[:, ct, :], wo_d[ct])

    nc.vector.memset(v[:, :, :, D:D + 1], 1.0)  # softmax-denominator ones col

    with (
        tc.tile_pool(name="pst", bufs=2, space="PSUM") as pst,
        tc.tile_pool(name="ppv", bufs=2, space="PSUM") as ppv,
        tc.tile_pool(name="ppo", bufs=2, space="PSUM") as ppo,
        tc.tile_pool(name="pe", bufs=16) as pe_pool,
        tc.tile_pool(name="pnorm", bufs=2) as pnorm,
        tc.tile_pool(name="pout", bufs=4) as pout,
        tc.tile_pool(name="pdram", bufs=2, space="DRAM") as pdram,
    ):
        # ---- projection group emitters (each: one [P, QC] psum + copy) ----
        def emit_kproj(qc, mt):
            qsl = slice(qc * QC, (qc + 1) * QC)
            ps = ppo.tile([P, QC], f32, tag="po")
            for ko in range(KO):
                nc.tensor.matmul(
                    ps[:], wk[:, ko, mt * P:(mt + 1) * P], xt[:, ko, qsl],
                    start=(ko == 0), stop=(ko == KO - 1))
            nc.vector.tensor_copy(kt[:, mt, qsl], ps[:])

        def emit_qproj(qc, mt):
            qsl = slice(qc * QC, (qc + 1) * QC)
            ps = ppo.tile([P, QC], f32, tag="po")
            for ko in range(KO):
                nc.tensor.matmul(
                    ps[:], wq[:, ko, mt * P:(mt + 1) * P], xt[:, ko, qsl],
                    start=(ko == 0), stop=(ko == KO - 1))
            nc.vector.tensor_copy(qt[:, mt, qsl], ps[:])

        def emit_vproj(tt):
            tsl = slice(tt * P, (tt + 1) * P)
            ps = ppo.tile([P, QC], f32, tag="po")
            for ko in range(KO):
                nc.tensor.matmul(
                    ps[:], xt[:, ko, tsl], wv[:, ko, :],
                    start=(ko == 0), stop=(ko == KO - 1))
            nc.vector.tensor_copy(
                v[:, tt, :, 0:D], ps[:].rearrange("p (h d) -> p h d", d=D))

        if variant == "qkv":
            for mt in range(CH // P):
                for qc in range(NQC):
                    emit_kproj(qc, mt)
                    emit_qproj(qc, mt)
            for tt in range(NKT):
                emit_vproj(tt)
            ot = persist.tile([P, QC], f32, tag="dump")
            nc.vector.tensor_copy(
                ot[:].rearrange("p (h d) -> p h d", d=D), v[:, 0, :, 0:D])
            nc.sync.dma_start(out_d[0:P, 0:QC], ot[:])
            return

        # ---- serial prefix: just enough for the first head's S stream ----
        for qc in range(NQC):
            emit_kproj(qc, 0)
        emit_qproj(0, 0)

        # ---- deferred projection drain, paced through the first qc ----
        # item idx i covers (qc=i//64, h=(i//8)%8, g=i%8); S(h) needs
        # kt/qt mt=h//2; PV(item i, g) needs v tiles 2g, 2g+1.
        drain_at = {}
        for i in range(NG):               # V pairs feed PV of items 0..7
            drain_at[i] = [lambda tt=2 * i: emit_vproj(tt),
                           lambda tt=2 * i + 1: emit_vproj(tt)]
        for mt in range(1, CH // P):      # K/Q for heads 2mt..2mt+1 at idx 16mt
            base = 8 * mt
            for qc in range(NQC):
                drain_at.setdefault(base + qc, []).append(
                    lambda qc=qc, mt=mt: emit_kproj(qc, mt))
            drain_at.setdefault(base + 4, []).append(
                lambda mt=mt: emit_qproj(0, mt))
        # Q(qc+1) during qc, one mt per head boundary
        for qc in range(NQC - 1):
            for mt in range(CH // P):
                drain_at.setdefault(qc * 64 + mt * 8 + 6, []).append(
                    lambda qc=qc, mt=mt: emit_qproj(qc + 1, mt))

        def emit_outproj_group(qc, i):
            qt_i, nt = 4 * qc + i // 2, i % 2
            po = ppo.tile([P, QC], f32, tag="po")
            for ct in range(CH // P):
                nc.tensor.matmul(
                    po[:], ctxt[:, ct, qt_i * P:(qt_i + 1) * P],
                    wo[:, ct, nt * QC:(nt + 1) * QC],
                    start=(ct == 0), stop=(ct == CH // P - 1))
            ot = pout.tile([P, QC], f32, tag="ot")
            nc.vector.tensor_copy(ot[:], po[:])
            nc.sync.dma_start(
                out_d[qt_i * P:(qt_i + 1) * P, nt * QC:(nt + 1) * QC], ot[:])

        if variant != "attn":
            # out-projection of qc spread across the back half of qc+1
            for qc in range(NQC - 1):
                for i in range(8):
                    drain_at.setdefault((qc + 1) * 64 + 18 + i * 5, []).append(
                        lambda qc=qc, i=i: emit_outproj_group(qc, i))

        flat = [(qc, h, g)
                for qc in range(NQC) for h in range(NH) for g in range(NG)]
        st_q = []

        def emit_S(it):
            qc, h, g = it
            qsl = slice(qc * QC, (qc + 1) * QC)
            hp, b64 = h // 2, (h % 2) * D
            stS = pst.tile([P, 2, QC], f32, tag="st")
            for j in range(2):
                ik = 2 * g + j
                ksl = slice(ik * P, (ik + 1) * P)
                nc.tensor.matmul(
                    stS[:, j, :],
                    kt[b64:b64 + D, hp, ksl],
                    qt[b64:b64 + D, hp, qsl],
                    start=True, stop=True, tile_position=(b64, 0))
            st_q.append(stS)

        emit_S(flat[0])
        emit_S(flat[1])
        post_loop = {}

        eS_hist = {}   # glob item idx -> eS tile

        def emit_pv_sweep(qc, h, qs, psc):
            # one sequentially-closed psum accumulation group per q-subtile
            glob = qc * 64 + h * 8
            for g in range(NG):
                eS = eS_hist[glob + g]
                for j in range(2):
                    nc.tensor.matmul(
                        psc[:, qs, :],
                        eS[:, j, qs * P:(qs + 1) * P],
                        v[:, 2 * g + j, h, 0:D + 1],
                        start=(g == 0 and j == 0),
                        stop=(g == NG - 1 and j == 1))

        def emit_normalize(qc, h, psc):
            qsl = slice(qc * QC, (qc + 1) * QC)
            rec = pnorm.tile([P, 4, 1], f32, tag="rec")
            nc.vector.reciprocal_approx_fast(rec[:], psc[:, :, D:D + 1])
            ctxn = pnorm.tile([P, 4, D], bf16, tag="ctxn")
            nc.vector.tensor_mul(ctxn[:], psc[:, :, 0:D],
                                 rec[:].broadcast_to([P, 4, D]))
            return ctxn

        def emit_transpose(qc, h, ctxn):
            qsl = slice(qc * QC, (qc + 1) * QC)
            tr = ppv.tile([P, QC], bf16, tag="tr", bufs=1)
            for qs in range(4):
                nc.tensor.transpose(tr[0:D, qs * P:(qs + 1) * P],
                                    ctxn[:, qs, :], eye[:])
            nc.vector.tensor_copy(
                ctxt[(h % 2) * D:(h % 2 + 1) * D, h // 2, qsl], tr[0:D, :])

        def defer(idx, fn):
            if idx < len(flat):
                drain_at.setdefault(idx, []).append(fn)
            else:
                post_loop.setdefault(idx, []).append(fn)

        for idx, it in enumerate(flat):
            qc, h, g = it
            glob = qc * 64 + h * 8
            stS = st_q.pop(0)
            eS = pe_pool.tile([P, 2, QC], bf16, tag="eS")
            eS_hist[idx] = eS
            nc.scalar.activation(eS[:], stS[:],
                                 mybir.ActivationFunctionType.Exp, scale=SCALE)
            for fn in drain_at.pop(idx, ()):
                fn()
            if idx + 2 < len(flat):
                emit_S(flat[idx + 2])
            if g == 0:
                # schedule this head's PV sweeps / normalize / transpose
                # relative to the end of its exp stream
                psc = ppv.tile([P, 4, D + 1], f32, tag="psc", bufs=1)
                for qs in range(4):
                    defer(glob + 8 + 2 * qs + 1,
                          lambda qc=qc, h=h, qs=qs, psc=psc:
                          emit_pv_sweep(qc, h, qs, psc))
                state = {}

                def norm_fn(qc=qc, h=h, psc=psc, state=state):
                    state["ctxn"] = emit_normalize(qc, h, psc)

                def tr_fn(qc=qc, h=h, state=state):
                    emit_transpose(qc, h, state["ctxn"])
                defer(glob + 16, norm_fn)
                defer(glob + 19, tr_fn)

        for idx in sorted(post_loop):
            for fn in post_loop[idx]:
                fn()
        if variant != "attn":
            for i in range(8):
                emit_outproj_group(NQC - 1, i)

        if variant == "attn":
            ot = persist.tile([P, QC], f32, tag="dump")
            nc.vector.tensor_copy(ot[:], ctxt[:, 0, 0:QC])
            nc.sync.dma_start(out_d[0:P, 0:QC], ot[:])


def _prepare_in_maps(x, wq, wk, wv, wo):
    x = np.asarray(x, dtype=np.float32)
    ws = {}
    for hg in range(HG):
        sl = slice(hg * CH, (hg + 1) * CH)
        ws[hg] = {
            "wq": np.ascontiguousarray(np.asarray(wq)[sl, :].T).astype(
                np_bf16).reshape(KO, P, CH),
            "wk": np.ascontiguousarray(np.asarray(wk)[sl, :].T).astype(
                np_bf16).reshape(KO, P, CH),
            "wv": np.ascontiguousarray(np.asarray(wv)[sl, :].T).astype(
                np_bf16).reshape(KO, P, CH),
            "wo": np.ascontiguousarray(np.asarray(wo)[:, sl].T).astype(
                np_bf16).reshape(CH // P, P, C),
        }
    eye = np.eye(P, dtype=np_bf16)
    in_maps = []
    for core in range(8):
        b, hg = core // HG, core % HG
        xtb = np.ascontiguousarray(x[b].T).astype(np_bf16).reshape(KO, P, N)
        m = {"xt": xtb, "eye": eye}
        m.update(ws[hg])
        in_maps.append(m)
    return in_maps


def kernel(x, wq, wk, wv, wo, bo):
    nc = _build()
    in_maps = _prepare_in_maps(x, wq, wk, wv, wo)
    res = run_bass_kernel_spmd(nc, in_maps, core_ids=list(range(8)))
    bo = np.asarray(bo, dtype=np.float32)
    out = np.empty((B, N, C), dtype=np.float32)
    for b in range(B):
        out[b] = res.results[2 * b]["out"] + res.results[2 * b + 1]["out"] + bo
    return out
